# revision 1
# baseline (speedup 1.0000x reference)
"""Expert-parallel MoE MLP (ExpertMLP) Bass kernel for 8 Trainium2 NeuronCores.

Problem: x[32,4096,256] @ w_fc[32,256,1024] -> gelu(erf) -> @ w_proj[32,1024,256].

Sharding: expert-parallel. Each of the 8 cores gets 4 experts (slices of the
leading axis of every tensor); no cross-core communication. Inside a core, per
expert e:

  1. x[e] ([4096,256], capacity-major) is transposed on the PE (identity
     matmul, 128x128 blocks) into xT [d, c] so the d-contraction of the first
     matmul lies on the partition axis.
  2. MM1: hT[h_tile, c_chunk] += w_fc_tile.T @ xT_chunk - w_fc's natural
     [d, h] layout is the stationary operand, so it needs no transpose.
  3. GELU (exact erf form) runs on the ACT engine as the PSUM->SBUF eviction.
  4. MM2 uses hT slices as the *stationary* operand and w_proj's natural
     [h, d] layout as the moving operand: out[c_sub, d] += hT_slice.T @
     w_proj_tile. The result lands directly in [capacity, d] orientation, so
     no output transpose is needed.

All matmul operands are float32r (e8m11, 1 PE cycle/row at N>=256 vs 4 for
fp32); producers (DVE copies / ACT gelu) write f32r tiles, which performs the
required rounding. PSUM accumulation stays fp32.
"""

import numpy as np
from contextlib import ExitStack

import bass_rust as _br
import concourse.bass as bass
import concourse.tile as tile
from concourse import mybir
from concourse.bass_utils import run_bass_kernel_spmd
from concourse.masks import make_identity

E, CAP, D, H = 32, 4096, 256, 1024
N_CORES = 8
E_PER = E // N_CORES  # 4 experts per core
P = 128
F32 = mybir.dt.float32
F32R = mybir.dt.float32r
BF16 = mybir.dt.bfloat16

KD = D // P        # 2 k-tiles in MM1's contraction
KH = H // P        # 8 k-tiles in MM2's contraction
NC_CHUNK = 512     # capacity chunk processed per MM1/MM2 round
N_CHUNKS = CAP // NC_CHUNK
H_TILES = H // P
C_TILES = CAP // P


def _fix_waits(nc):
    """walrus here accepts only one sync wait per instruction; hoist excess
    waits onto standalone EventSemaphore instructions inserted before the
    offender (same engine => same sequencer order)."""
    for fn in nc.m.functions:
        for bb in fn.blocks:
            new = []
            changed = False
            for inst in bb.instructions:
                si = inst.sync_info
                if si is not None and len(si.on_wait) > 1:
                    waits = list(si.on_wait)
                    for w in waits[:-1]:
                        ev = mybir.InstEventSemaphore(
                            name=nc.get_next_instruction_name()
                        )
                        ev.engine = inst.engine
                        ev.sync_info = _br.SyncInfo(on_wait=[w], on_update=[])
                        nc.register_instruction(ev)
                        new.append(ev)
                    inst.sync_info = _br.SyncInfo(
                        on_wait=waits[-1:], on_update=list(si.on_update)
                    )
                    changed = True
                new.append(inst)
            if changed:
                bb.instructions = new


def _build():
    nc = bass.Bass(trn_type="TRN2", target_bir_lowering=False, debug=False)
    x = nc.dram_tensor("x", [E_PER, CAP, D], F32, kind="ExternalInput").ap()
    w_fc = nc.dram_tensor("w_fc", [E_PER, D, H], F32, kind="ExternalInput").ap()
    w_proj = nc.dram_tensor("w_proj", [E_PER, H, D], F32, kind="ExternalInput").ap()
    out = nc.dram_tensor("out", [E_PER, CAP, D], F32, kind="ExternalOutput").ap()
    # bf16 staging copies of x so the XBar DMA-transpose (2-byte dtype only)
    # can build xT without burning TensorE cycles on identity transposes.
    # One DRAM tensor per (expert, half): DRAM dependency tracking is
    # tensor-granular, so finer tensors let each transpose start as soon as
    # its own cast chunk lands instead of after all casts.
    CASTCH = CAP // 2  # cast-DMA chunk (rows)
    xbf = [
        [
            nc.dram_tensor(f"xbf{e}_{hh}", [CASTCH, D], BF16).ap()
            for hh in range(CAP // CASTCH)
        ]
        for e in range(E_PER)
    ]

    with tile.TileContext(nc) as tc, ExitStack() as ctx:
        xtp = ctx.enter_context(tc.tile_pool(name="xtp", bufs=2 * E_PER))
        wload = ctx.enter_context(tc.tile_pool(name="wload", bufs=2))
        wfc_p = ctx.enter_context(tc.tile_pool(name="wfc", bufs=2))
        wproj_p = ctx.enter_context(tc.tile_pool(name="wproj", bufs=2))
        ht_p = ctx.enter_context(tc.tile_pool(name="ht", bufs=8))
        out_p = ctx.enter_context(tc.tile_pool(name="outp", bufs=3))
        ps_h = ctx.enter_context(tc.tile_pool(name="ps_h", bufs=2, space="PSUM"))
        ps_o = ctx.enter_context(tc.tile_pool(name="ps_o", bufs=4, space="PSUM"))

        HPACK = 2          # h_tiles packed per PSUM tile / GELU call
        SLAB = 1024        # DMA-transpose slab (capacity columns)

        def load_weights(e):
            wfc_raw = wload.tile([P, KD, H], F32, tag="wl")
            nc.sync.dma_start(wfc_raw[:], w_fc[e].rearrange("(k p) h -> p k h", p=P))
            wfc = wfc_p.tile([P, KD, H], BF16, tag="wfc")
            nc.vector.tensor_copy(wfc[:], wfc_raw[:])
            wproj_raw = wload.tile([P, KH, D], F32, tag="wl")
            nc.sync.dma_start(
                wproj_raw[:], w_proj[e].rearrange("(k p) d -> p k d", p=P)
            )
            wproj = wproj_p.tile([P, KH, D], BF16, tag="wproj")
            nc.vector.tensor_copy(wproj[:], wproj_raw[:])
            return wfc, wproj

        # ---- prologue: expert 0's weights first, then stage all experts' xT:
        # DRAM->DRAM cast x[e]->bf16 in half-chunks (q0 FIFO => e0 first),
        # then XBar-transpose 1024-column slabs into SBUF on the scalar HWDGE
        # queue so they don't queue behind weight/output traffic on q1.
        # MM1 of (e, chunk) only needs its slab, so compute starts early.
        w0 = load_weights(0)
        for e in range(E_PER):
            for hh in range(CAP // CASTCH):
                rs = slice(hh * CASTCH, (hh + 1) * CASTCH)
                nc.gpsimd.dma_start(xbf[e][hh][:], x[e][rs])
        SPH = CASTCH // SLAB  # slabs per cast half
        xts = []
        for e in range(E_PER):
            xt = [
                [
                    xtp.tile([P, SLAB], BF16, tag="xt", name=f"xt{e}_{k}_{s}")
                    for s in range(CAP // SLAB)
                ]
                for k in range(KD)
            ]
            for s in range(CAP // SLAB):
                ls = slice((s % SPH) * SLAB, (s % SPH + 1) * SLAB)
                for k in range(KD):
                    nc.sync.dma_start_transpose(
                        xt[k][s][:], xbf[e][s // SPH][ls, k * P:(k + 1) * P]
                    )
            xts.append(xt)

        for e in range(E_PER):
            xt = xts[e]
            wfc, wproj = w0 if e == 0 else load_weights(e)

            # ---- MM1 -> GELU -> MM2 per capacity chunk ----
            # MM1 accumulates HPACK h_tiles into one multi-bank PSUM tile so
            # GELU evicts in wider (cheaper) ACTIVATE calls; hT is written in
            # bf16 so MM2's per-matmul weight loads run at 2-byte FWL speed.
            for nci in range(N_CHUNKS):
                csl = slice(nci * NC_CHUNK, (nci + 1) * NC_CHUNK)
                ht_tiles = []  # HPACK-wide bf16 tiles
                for hp in range(H_TILES // HPACK):
                    psh = ps_h.tile([P, HPACK, NC_CHUNK], F32, tag="psh")
                    for j in range(HPACK):
                        hi = hp * HPACK + j
                        for k in range(KD):
                            sidx = (nci * NC_CHUNK) // SLAB
                            soff = (nci * NC_CHUNK) % SLAB
                            nc.tensor.matmul(
                                psh[:, j, :],
                                wfc[:, k, hi * P:(hi + 1) * P],
                                xt[k][sidx][:, soff:soff + NC_CHUNK],
                                start=(k == 0),
                                stop=(k == KD - 1),
                            )
                    ht = ht_p.tile([P, HPACK, NC_CHUNK], BF16, tag="ht")
                    nc.scalar.activation(
                        ht[:], psh[:], mybir.ActivationFunctionType.Gelu
                    )
                    ht_tiles.append(ht)

                ob = out_p.tile([P, NC_CHUNK // P, D], F32, tag="ob")
                for s in range(NC_CHUNK // P):
                    pso = ps_o.tile([P, D], F32, tag="pso")
                    for k in range(KH):
                        nc.tensor.matmul(
                            pso[:],
                            ht_tiles[k // HPACK][:, k % HPACK, s * P:(s + 1) * P],
                            wproj[:, k, :],
                            start=(k == 0),
                            stop=(k == KH - 1),
                        )
                    nc.vector.tensor_copy(ob[:, s, :], pso[:])
                nc.sync.dma_start(
                    out[e, csl, :].rearrange("(s p) d -> p s d", p=P), ob[:]
                )

    _fix_waits(nc)
    return nc


_CACHE = {}


def _get_nc():
    if "nc" not in _CACHE:
        _CACHE["nc"] = _build()
    return _CACHE["nc"]


def kernel(x, w_fc, w_proj, trace=False):
    assert x.shape == (E, CAP, D) and w_fc.shape == (E, D, H)
    assert w_proj.shape == (E, H, D)
    nc = _get_nc()
    x = np.ascontiguousarray(x, dtype=np.float32)
    w_fc = np.ascontiguousarray(w_fc, dtype=np.float32)
    w_proj = np.ascontiguousarray(w_proj, dtype=np.float32)
    in_maps = [
        {
            "x": x[i * E_PER:(i + 1) * E_PER],
            "w_fc": w_fc[i * E_PER:(i + 1) * E_PER],
            "w_proj": w_proj[i * E_PER:(i + 1) * E_PER],
        }
        for i in range(N_CORES)
    ]
    res = run_bass_kernel_spmd(nc, in_maps, list(range(N_CORES)), trace=trace)
    out = np.concatenate([r["out"] for r in res.results], axis=0)
    if trace:
        kernel.last_results = res
    return out

